# revision 14
# baseline (speedup 1.0000x reference)
"""ANI AEV representation kernel for 8 Trainium2 NeuronCores.

Strategy (data-parallel over atoms, per the sharding hint):
  - Atoms are partitioned into 8 contiguous shards of 6250.
  - Each core computes its (6250, 1008) slice of the AEV in a flat
    "slot" layout: radial slots (atom,species) -> 16 floats, angular
    slots (atom,species-pair) -> 32 floats.
  - Host planning sorts every scatter contribution by destination slot.
    The first contribution of every slot (or an inert pad) forms a dense
    "base image" that is written to DRAM at full bandwidth.  Remaining
    contributions ("extras") are grouped by exact per-slot count, summed
    on-chip with strided adds, and applied with dma_scatter_add (CCE
    f32 accumulate in the DMA datapath).
  - All transcendentals use one ACT table set (exp/ln/square); the
    cosine cutoff is a degree-3 polynomial in d^2, division is
    exp(-ln(x)), and cos(theta - z_j) is expanded so no trig is needed.
"""

import sys

sys.path.insert(0, "/opt/trn_rl_repo")

import numpy as np

import concourse.bass as bass
import concourse.bass_isa as bass_isa
import concourse.mybir as mybir
from concourse import library_config
from concourse.library_overlay import lower_extended_insts
from concourse.bass_utils import run_bass_kernel_spmd
from concourse.tile import TileContext

# ---- problem constants (must match reference.py) ----
N = 50000
NCORE = 8
NB = N // NCORE          # 6250 atoms per core
S = 7
NRBF = 16
RC = 0.51
RMIN = 0.08
RCA = 0.35
RAMIN = 0.08
NA = 8
NZ = 4
ETA_R = 1970.0
ETA_A = 1250.0
ZETA = 14.1
NPAIRS = S * (S + 1) // 2   # 28
SUB = NA * NZ               # 32

RSLOTS = NB * S             # 43750 radial slots per core
ASLOTS = NB * NPAIRS        # 175000 angular slots per core
OUT_LEN = RSLOTS * NRBF + ASLOTS * SUB   # 6,300,000 = 6250*1008
AOFF = RSLOTS * NRBF        # angular region offset in flat out

ECAP_R = 8                  # extras chunk cap, radial
ECAP_A = 4                  # extras chunk cap, angular
RPHASES = 4                 # radial slot stride phases (256B alignment)
APHASES = 2                 # angular phases
AWIN = 32768                # angular scatter window rows (int16 idx limit)
MBLK = 125                  # base-superblock free columns per partition

F32 = mybir.dt.float32
I16 = mybir.dt.int16
AF = mybir.ActivationFunctionType
OP = mybir.AluOpType

CENTERS_R = (RMIN + (RC - RMIN) / NRBF * np.arange(NRBF)).astype(np.float64)
SHFA = (RAMIN + (RCA - RAMIN) / NA * np.arange(NA)).astype(np.float64)
SHFZ = ((np.arange(NZ) + 0.5) * (np.pi / NZ)).astype(np.float64)
COSZ = np.cos(SHFZ)
SINZ = np.sin(SHFZ)

# degree-3 polynomial for cos(sqrt(u)), u in [0, (pi/2)^2]; fc = cos^2(y).
# p(u) = C3*(((u + E2)*u + E1)*u + E0)  evaluated with stt/ts ops.
_u = np.linspace(0, (np.pi / 2) ** 2, 2001)
_c = np.polynomial.chebyshev.Chebyshev.fit(_u, np.cos(np.sqrt(_u)), deg=3)
_p = _c.convert(kind=np.polynomial.Polynomial)
_c0, _c1, _c2, _c3 = [float(x) for x in _p.coef]
COSP_E2 = _c2 / _c3
COSP_E1 = _c1 / _c3
COSP_E0 = _c0 / _c3
COSP_C3 = _c3


def _triu_index_np(num_species):
    s1, s2 = np.triu_indices(num_species)
    ret = np.zeros((num_species, num_species), dtype=np.int64)
    ret[s1, s2] = np.arange(len(s1))
    ret[s2, s1] = np.arange(len(s1))
    return ret


TRIU = _triu_index_np(S)

# --------------------------------------------------------------------------
# Host planning
# --------------------------------------------------------------------------


def _segment(slots, nslots):
    """Sort by slot; return sorted order, counts, starts."""
    order = np.argsort(slots, kind="stable")
    ss = slots[order]
    counts = np.bincount(ss, minlength=nslots)
    starts = np.zeros(nslots + 1, dtype=np.int64)
    np.cumsum(counts, out=starts[1:])
    return order, ss, counts, starts


def _plan_side(slots, rows, nslots, ecap, inert_row):
    """Plan one side. Returns base image + raw extras (slot, rank, rows)."""
    w = rows.shape[1]
    order, ss, counts, starts = _segment(slots, nslots)
    rows_s = rows[order]

    base = np.tile(inert_row.astype(np.float32), (nslots, 1))
    nz = counts > 0
    base[nz] = rows_s[starts[:-1][nz]]

    npos = len(ss)
    rank = np.arange(npos) - np.repeat(starts[:-1], counts)
    ex = rank >= 1
    e_total = (counts - 1).clip(0)
    return base, (ss[ex], rank[ex] - 1, rows_s[ex], e_total)


RBUCKETS = [1, 2, 3, 4, 5, 6, 7, 8, 10, 12, 14, 16, 20, 24, 28, 32, 40,
            48, 64, 96, 128]
ABUCKETS = [1, 2, 3, 4, 6, 8, 12, 16, 24, 32, 48, 64, 96, 128]


def _bucketize(e, buckets):
    b = np.asarray(buckets)
    return b[np.searchsorted(b, e)]


def _build_sections(ex, nslots, buckets, nphase, win, w, inert_row):
    """Group extras into (phase, window, e-bucket) sections; each slot
    appears in exactly one section with its count padded to a bucket."""
    ex_slot, ex_rank, ex_rows, e_total = ex
    out = {}
    if len(ex_slot) == 0:
        return out
    eb_all = np.zeros(nslots, np.int64)
    have = e_total > 0
    eb_all[have] = _bucketize(e_total[have], buckets)
    eb = eb_all[ex_slot]
    phase = ex_slot % nphase
    window = (ex_slot // nphase) // win
    sidx = np.lexsort((ex_rank, ex_slot, eb, window, phase))
    ph, wi, cs, sl, po, rw = (phase[sidx], window[sidx], eb[sidx],
                              ex_slot[sidx], ex_rank[sidx], ex_rows[sidx])
    keyarr = np.stack([ph, wi, cs])
    change = np.any(keyarr[:, 1:] != keyarr[:, :-1], axis=0)
    bounds = np.concatenate([[0], np.nonzero(change)[0] + 1, [len(sl)]])
    for a, b in zip(bounds[:-1], bounds[1:]):
        P_, W_, E_ = int(ph[a]), int(wi[a]), int(cs[a])
        sls, pos, rws = sl[a:b], po[a:b], rw[a:b]
        uslot, uinv = np.unique(sls, return_inverse=True)
        block = np.tile(inert_row.astype(np.float32),
                        (len(uslot), E_, 1))
        block[uinv, pos] = rws
        out[(P_, W_, E_)] = (uslot, block)
    return out


def _plan_core(core, contrib_r, contrib_a):
    base = core * NB
    # ---------------- radial ----------------
    dest, osp, dval = contrib_r
    m = (dest >= base) & (dest < base + NB)
    slot_r = ((dest[m] - base) * S + osp[m]).astype(np.int64)
    rows_r = dval[m].astype(np.float32)[:, None]
    inert_r = np.array([1.0], np.float32)
    base_r, ex_r = _plan_side(slot_r, rows_r, RSLOTS, ECAP_R, inert_r)
    sec_r = _build_sections(ex_r, RSLOTS, RBUCKETS, RPHASES, 1 << 30, 1,
                            inert_r)

    # ---------------- angular ----------------
    cent, cls, geom = contrib_a
    m = (cent >= base) & (cent < base + NB)
    slot_a = ((cent[m] - base) * NPAIRS + cls[m]).astype(np.int64)
    rows_a = geom[m].astype(np.float32)
    inert = np.array([1, 0, 0, 1, 1, 0, 0, 1], np.float32)
    base_a, ex_a = _plan_side(slot_a, rows_a, ASLOTS, ECAP_A, inert)
    sec_a = _build_sections(ex_a, ASLOTS, ABUCKETS, APHASES, AWIN, 8, inert)

    return dict(base_r=base_r, sec_r=sec_r, base_a=base_a, sec_a=sec_a)


def _common_sections(plans, key):
    """Union of section keys with max padded position counts across cores."""
    keys = set()
    for p in plans:
        keys.update(p[key].keys())
    table = []
    for k in sorted(keys):
        n = max(len(p[key][k][0]) if k in p[key] else 0 for p in plans)
        n_pad = ((n + 127) // 128) * 128
        table.append((k, n_pad))
    return table


def _pack_core(plan, key, table, w, nphase, win_rows, view_rows_fn):
    """Flat data array, idx array, and per-(phase,window) call list.

    Device per-call tile: (128, E_call, w) where partition p's row is the
    concat of all sections' entries for that partition.  Within a section
    (m, e): scatter position q=(mm*128+p) stored at host row (q%128)*m+q//128.
    """
    secs = plan[key]
    calls = {}
    for (k, n_pad) in table:
        P_, W_, E_ = k
        calls.setdefault((P_, W_), []).append((E_, n_pad))

    data_parts = []
    idx_parts = []
    call_descs = []
    data_off = 0
    for (P_, W_), classes in sorted(calls.items()):
        ntot = sum(n for _, n in classes)
        idx_full = np.zeros(ntot, np.int16)
        sec_descs = []
        qoff = 0
        part_rows = []   # per-section (128, m*e, w)
        for (E_, n_pad) in classes:
            m = n_pad // 128
            k = (P_, W_, E_)
            if k in secs:
                uslot, block = secs[k]
            else:
                uslot = np.zeros(0, np.int64)
                block = np.zeros((0, E_, w), np.float32)
            n = len(uslot)
            fullb = np.zeros((n_pad, E_, w), np.float32)
            fullb[:n] = block
            if w == 1:
                fullb[n:] = 1.0
            else:
                fullb[n:] = np.array([1, 0, 0, 1, 1, 0, 0, 1], np.float32)
            idx = np.zeros(n_pad, np.int16)
            idx[:n] = (uslot // nphase) - W_ * win_rows
            q = np.arange(n_pad)
            host_pos = (q % 128) * m + q // 128
            perm = np.zeros_like(fullb)
            perm[host_pos] = fullb
            part_rows.append(perm.reshape(128, m * E_, w))
            idx_full[qoff:qoff + n_pad] = idx
            sec_descs.append(dict(e=E_, m=m, n_pad=n_pad))
            qoff += n_pad
        callblk = np.concatenate(part_rows, axis=1)   # (128, E_call, w)
        data_parts.append(callblk.reshape(-1))
        it = np.zeros((128, ntot // 16), np.int16)
        for rep in range(8):
            it[rep * 16:(rep + 1) * 16, :] = idx_full.reshape(-1, 16).T
        idx_parts.append(it)
        call_descs.append(dict(phase=P_, window=W_, ntot=ntot,
                               sections=sec_descs, data_off=data_off,
                               e_call=callblk.shape[1],
                               view_rows=view_rows_fn(P_, W_)))
        data_off += callblk.size
    data = (np.concatenate(data_parts) if data_parts
            else np.zeros(1, np.float32))
    idx = (np.concatenate(idx_parts, axis=1) if idx_parts
           else np.zeros((128, 1), np.int16))
    return data, idx, call_descs


def _prepare(inputs):
    """Full host planning: returns in_maps and the static call layout."""
    atom_index = np.asarray(inputs["atom_index"])
    pair_indices = np.asarray(inputs["pair_indices"])
    d_ij = np.asarray(inputs["d_ij"])
    r_ij = np.asarray(inputs["r_ij"])
    central = np.asarray(inputs["central_atom_index"])
    p12 = np.asarray(inputs["pair_index12"])
    sign12 = np.asarray(inputs["sign12"])

    i, j = pair_indices[0], pair_indices[1]
    si, sj = atom_index[i], atom_index[j]
    d = d_ij[:, 0].astype(np.float32)
    dest = np.concatenate([i, j])
    osp = np.concatenate([sj, si]).astype(np.int64)
    dval = np.concatenate([d, d])
    contrib_r = (dest, osp, dval)

    p0, p1 = p12[0], p12[1]
    sgn0 = sign12[0].astype(np.float32)
    sgn1 = sign12[1].astype(np.float32)
    v0 = r_ij[p0] * sgn0[:, None]
    v1 = r_ij[p1] * sgn1[:, None]
    d0 = d[p0]
    d1 = d[p1]
    s0 = np.where(sign12[0] == 1, sj[p0], si[p0])
    s1 = np.where(sign12[1] == 1, sj[p1], si[p1])
    cls = TRIU[s0, s1].astype(np.int64)
    geom = np.concatenate(
        [v0, d0[:, None], v1, d1[:, None]], axis=1).astype(np.float32)
    contrib_a = (central, cls, geom)

    plans = [_plan_core(c, contrib_r, contrib_a) for c in range(NCORE)]

    table_r = _common_sections(plans, "sec_r")
    table_a = _common_sections(plans, "sec_a")

    rview = lambda P_, W_: (RSLOTS - P_ + RPHASES - 1) // RPHASES
    aview = lambda P_, W_: min(AWIN, ASLOTS // APHASES - W_ * AWIN)

    in_maps = []
    layout = None
    for c in range(NCORE):
        p = plans[c]
        data_r, idx_r, calls_r = _pack_core(
            p, "sec_r", table_r, 1, RPHASES, 1 << 30, rview)
        data_a, idx_a, calls_a = _pack_core(
            p, "sec_a", table_a, 8, APHASES, AWIN, aview)
        if layout is None:
            layout = dict(calls_r=calls_r, calls_a=calls_a,
                          len_dr=data_r.size, len_da=data_a.size,
                          len_ir=idx_r.shape[1], len_ia=idx_a.shape[1])
        in_maps.append({
            "d_base": np.ascontiguousarray(p["base_r"][:, 0]),
            "g_base": np.ascontiguousarray(p["base_a"].reshape(-1)),
            "d_ext": data_r, "g_ext": data_a,
            "idx_r": idx_r, "idx_a": idx_a,
        })
    return in_maps, layout


# --------------------------------------------------------------------------
# Device kernel builder
# --------------------------------------------------------------------------


def _emit_cutoff_sq(nc, pool, d_ap, n, halfpi_scale, out_name):
    """Returns tile (128p, n) with q = 0.5*cos(pi*d/(2*rc)*2) ... actually
    returns qh = 0.5*cos(y) approx where y = d*halfpi_scale (in [0, pi/2]).
    Caller squares it (ACT Square) to get 0.25*fc or combines two.
    Partition count follows d_ap."""
    P_ = d_ap.shape[0]
    u = pool.tile([P_, n], F32, tag=out_name + "_u", name=out_name + "_u")
    nc.scalar.activation(u[:, :], d_ap, AF.Square, scale=float(halfpi_scale))
    h = pool.tile([P_, n], F32, tag=out_name + "_h", name=out_name + "_h")
    nc.vector.scalar_tensor_tensor(h[:, :], u[:, :], COSP_E2, u[:, :],
                                   OP.add, OP.mult)
    nc.vector.scalar_tensor_tensor(h[:, :], h[:, :], COSP_E1, u[:, :],
                                   OP.add, OP.mult)
    # qh = 0.5*C3*(h + E0) = 0.5*cos(y)
    nc.vector.tensor_scalar(h[:, :], h[:, :], 0.5 * COSP_C3,
                            0.5 * COSP_C3 * COSP_E0, OP.mult, OP.add)
    return h


def _emit_radial_terms(nc, pool, dtile, P_, E, out_ap, tag):
    """dtile (P_, E) distances -> out_ap (P_, E, NRBF) terms (f32).

    term_r = 0.25*fc(d)*exp(-eta*(d-c_r)^2);  0.25*fc = Square(qh)."""
    qh = _emit_cutoff_sq(nc, pool, dtile[:, :], E, np.pi / (2 * RC), tag + "q")
    fc4 = pool.tile([P_, E], F32, tag=tag + "fc", name=tag + "fc")
    nc.scalar.activation(fc4[:, :], qh[:, :], AF.Square)
    for r in range(NRBF):
        nc.scalar.activation(out_ap[:, :, r], dtile[:, :], AF.Square,
                             bias=-float(CENTERS_R[r]))
    nc.scalar.activation(out_ap, out_ap, AF.Exp, scale=-ETA_R)
    nc.vector.tensor_tensor(
        out_ap, out_ap,
        fc4[:, :].unsqueeze(2).broadcast_to([P_, E, NRBF]), OP.mult)


def _emit_angular_terms(nc, pool, g, P_, E, out_ap, tag):
    """g (P_, E, 8) geometry -> out_ap (P_, E, SUB) terms.

    Layout of 32: w-major (NA=8), j-minor (NZ=4): term[.,w,j]."""
    x0, y0, z0, d0 = (g[:, :, k] for k in range(4))
    x1, y1, z1, d1 = (g[:, :, k] for k in range(4, 8))
    t_ = lambda nm: pool.tile([P_, E], F32, tag=tag + nm, name=tag + nm)

    dot = t_("dot"); tmp = t_("tmp")
    nc.vector.tensor_tensor(dot[:, :], x0, x1, OP.mult)
    nc.vector.tensor_tensor(tmp[:, :], y0, y1, OP.mult)
    nc.vector.tensor_tensor(dot[:, :], dot[:, :], tmp[:, :], OP.add)
    nc.vector.tensor_tensor(tmp[:, :], z0, z1, OP.mult)
    nc.vector.tensor_tensor(dot[:, :], dot[:, :], tmp[:, :], OP.add)

    p01 = t_("p01")
    nc.vector.tensor_tensor(p01[:, :], d0, d1, OP.mult)
    nc.scalar.activation(p01[:, :], p01[:, :], AF.Ln)
    # rp = 0.95 / (d0*d1)
    nc.scalar.activation(p01[:, :], p01[:, :], AF.Exp, scale=-1.0,
                         bias=float(np.log(0.95)))
    mu = t_("mu")
    nc.vector.tensor_tensor(mu[:, :], dot[:, :], p01[:, :], OP.mult)

    msq = t_("msq")
    nc.scalar.activation(msq[:, :], mu[:, :], AF.Square)
    nc.vector.tensor_scalar(msq[:, :], msq[:, :], -1.0, 1.0, OP.mult, OP.add)
    nc.scalar.activation(msq[:, :], msq[:, :], AF.Ln)
    sig = t_("sig")
    nc.scalar.activation(sig[:, :], msq[:, :], AF.Exp, scale=0.5)

    # cutoffs: fc2 = 2*fc0*fc1 = 8*(qh0*qh1)^2
    qh0 = _emit_cutoff_sq(nc, pool, d0, E, np.pi / (2 * RCA), tag + "qa")
    qh1 = _emit_cutoff_sq(nc, pool, d1, E, np.pi / (2 * RCA), tag + "qb")
    nc.vector.tensor_tensor(qh0[:, :], qh0[:, :], qh1[:, :], OP.mult)
    fc2 = t_("fc2")
    nc.scalar.activation(fc2[:, :], qh0[:, :], AF.Square,
                         scale=float(np.sqrt(32.0)))

    # t_j = 0.5 + 0.5*cos(z_j)*mu + 0.5*sin(z_j)*sig  -> (P_, NZ, E)
    tj = pool.tile([P_, NZ, E], F32, tag=tag + "tj", name=tag + "tj")
    for jj in range(NZ):
        nc.vector.tensor_scalar(tj[:, jj, :], sig[:, :],
                                0.5 * float(SINZ[jj]), 0.5, OP.mult, OP.add)
        nc.vector.scalar_tensor_tensor(tj[:, jj, :], mu[:, :],
                                       0.5 * float(COSZ[jj]), tj[:, jj, :],
                                       OP.mult, OP.add)
    nc.vector.tensor_scalar(tj[:, :, :], tj[:, :, :], 1e-20, None,
                            OP.max, OP.bypass)
    nc.scalar.activation(tj[:, :, :], tj[:, :, :], AF.Ln)
    nc.scalar.activation(tj[:, :, :], tj[:, :, :], AF.Exp, scale=ZETA)
    # g_j = f1_j * fc2
    nc.vector.tensor_tensor(
        tj[:, :, :], tj[:, :, :],
        fc2[:, :].unsqueeze(1).broadcast_to([P_, NZ, E]), OP.mult)

    # f2_w = exp(-eta*(0.5*(d0+d1) - shfa_w)^2) -> (P_, NA, E)
    ds = t_("ds")
    nc.vector.tensor_tensor(ds[:, :], d0, d1, OP.add)
    f2 = pool.tile([P_, NA, E], F32, tag=tag + "f2", name=tag + "f2")
    for w in range(NA):
        nc.scalar.activation(f2[:, w, :], ds[:, :], AF.Square, scale=0.5,
                             bias=-float(SHFA[w]))
    nc.scalar.activation(f2[:, :, :], f2[:, :, :], AF.Exp, scale=-ETA_A)

    # term[e, w, j] = f2[w, e] * g[j, e]
    f2v = f2[:, :, :].transpose([0, 2, 1]).unsqueeze(3)      # (P_,E,NA,1)
    gv = tj[:, :, :].transpose([0, 2, 1]).unsqueeze(2)       # (P_,E,1,NZ)
    nc.vector.tensor_tensor(
        out_ap, f2v.broadcast_to([P_, E, NA, NZ]),
        gv.broadcast_to([P_, E, NA, NZ]), OP.mult)


def _blocks(total, m):
    """Split `total` rows into (offset, P, m) superblocks, P*m rows each."""
    out = []
    off = 0
    while off < total:
        rem = total - off
        if rem >= 128 * m:
            out.append((off, 128, m))
            off += 128 * m
        else:
            p = rem // m
            if p > 0:
                out.append((off, p, m))
                off += p * m
            if total - off > 0:
                out.append((off, 1, total - off))
                off = total
    return out


def build_nc(layout):
    import os
    no_scatter = os.environ.get("ANI_NO_SCATTER") == "1"
    no_extras = os.environ.get("ANI_NO_EXTRAS") == "1"
    nc = bass.Bass(num_swdge_queues=4)
    d_base = nc.declare_dram_parameter("d_base", [RSLOTS], F32, isOutput=False)
    g_base = nc.declare_dram_parameter("g_base", [ASLOTS * 8], F32, isOutput=False)
    d_ext = nc.declare_dram_parameter("d_ext", [layout["len_dr"]], F32, isOutput=False)
    g_ext = nc.declare_dram_parameter("g_ext", [layout["len_da"]], F32, isOutput=False)
    idx_r = nc.declare_dram_parameter("idx_r", [128, layout["len_ir"]], I16, isOutput=False)
    idx_a = nc.declare_dram_parameter("idx_a", [128, layout["len_ia"]], I16, isOutput=False)
    out = nc.declare_dram_parameter("out", [OUT_LEN], F32, isOutput=True)

    # activation bias operands must exist as (128,1) const tiles
    bias_vals = set()
    for r in range(NRBF):
        bias_vals.add(-float(CENTERS_R[r]))
    for w in range(NA):
        bias_vals.add(-float(SHFA[w]))
    bias_vals.add(float(np.log(0.95)))
    for k, v in enumerate(sorted(bias_vals)):
        t = nc.alloc_sbuf_tensor(f"bconst{k}", [128, 1], F32)
        nc.gpsimd.memset(t.ap(), v)
        nc.const_aps.aps[(F32, v)] = t.ap()
    nc.all_engine_barrier()

    with TileContext(nc) as tc:
        with tc.tile_pool(name="main", bufs=2) as pool, \
             tc.tile_pool(name="ext", bufs=1) as epool, \
             tc.tile_pool(name="sums", bufs=2) as spool, \
             tc.tile_pool(name="idxp", bufs=2) as ipool:

            # ---------------- radial base image ----------------
            for (off, P_, M_) in _blocks(RSLOTS, MBLK):
                dt = pool.tile([P_, M_], F32, tag="rb_d", name="rb_d")
                src = d_base[:].rearrange("(a) -> a")[off:off + P_ * M_] \
                    .rearrange("(p m) -> p m", p=P_)
                nc.sync.dma_start(out=dt[:, :], in_=src)
                terms = pool.tile([P_, M_, NRBF], F32, tag="rb_t", name="rb_t")
                _emit_radial_terms(nc, pool, dt, P_, M_, terms[:, :, :], "rb")
                dst = out[off * NRBF:(off + P_ * M_) * NRBF] \
                    .rearrange("(p m r) -> p m r", p=P_, m=M_)
                nc.sync.dma_start(out=dst, in_=terms[:, :, :])

            # ---------------- angular base image ----------------
            for (off, P_, M_) in _blocks(ASLOTS, MBLK):
                gt = pool.tile([P_, M_, 8], F32, tag="ab_g", name="ab_g")
                src = g_base[off * 8:(off + P_ * M_) * 8] \
                    .rearrange("(p m k) -> p m k", p=P_, m=M_)
                nc.sync.dma_start(out=gt[:, :, :], in_=src)
                terms = pool.tile([P_, M_, SUB], F32, tag="ab_t", name="ab_t")
                _emit_angular_terms(nc, pool, gt, P_, M_, terms[:, :, :], "ab")
                dst = out[AOFF + off * SUB:AOFF + (off + P_ * M_) * SUB] \
                    .rearrange("(p m r) -> p m r", p=P_, m=M_)
                nc.sync.dma_start(out=dst, in_=terms[:, :, :])

            # ---------------- extras + scatter ----------------
            self_state = {"q": 0}
            regcache = {}

            def emit_scatter(calls, data_param, idx_param, width, elem,
                             nphase, region_off, emit_terms, tagp):
                icol = 0
                for call in calls:
                    ntot = call["ntot"]
                    if ntot == 0 or no_extras:
                        continue
                    ecall = call["e_call"]
                    doff = call["data_off"]
                    dtile = epool.tile([128, ecall, width], F32,
                                      tag=tagp + "xd", name=tagp + "xd")
                    src_ap = data_param[doff:doff + 128 * ecall * width] \
                        .rearrange("(p q) -> p q", p=128)
                    nc.sync.dma_start(
                        out=dtile[:, :, :].rearrange("p a b -> p (a b)"),
                        in_=src_ap)
                    terms = epool.tile([128, ecall, elem], F32,
                                      tag=tagp + "xt", name=tagp + "xt")
                    if width == 1:
                        _emit_radial_terms(nc, epool, dtile[:, :, 0], 128,
                                           ecall, terms[:, :, :], tagp + "x")
                    else:
                        _emit_angular_terms(nc, epool, dtile, 128, ecall,
                                            terms[:, :, :], tagp + "x")
                    sums = spool.tile([128, ntot // 128, elem], F32,
                                      tag=tagp + "sums", name=tagp + "sums")
                    qbase = 0
                    eoff = 0
                    for sec in call["sections"]:
                        e, m = sec["e"], sec["m"]
                        ts = terms[:, eoff:eoff + m * e, :].rearrange(
                            "p (m e) r -> p m e r", m=m)
                        out_slice = sums[:, qbase:qbase + m, :]
                        if e == 1:
                            nc.vector.tensor_copy(out_slice, ts[:, :, 0, :])
                        elif e == 2:
                            nc.vector.tensor_tensor(
                                out_slice, ts[:, :, 0, :], ts[:, :, 1, :],
                                OP.add)
                        else:
                            nc.vector.tensor_reduce(
                                out_slice, ts.transpose([0, 1, 3, 2]),
                                axis=mybir.AxisListType.X, op=OP.add)
                        qbase += m
                        eoff += m * e
                    it = ipool.tile([128, ntot // 16], I16,
                                    tag=tagp + "idx", name=tagp + "idx")
                    nc.sync.dma_start(
                        out=it[:, :], in_=idx_param[:, icol:icol + ntot // 16])
                    icol += ntot // 16
                    nrows = call["view_rows"]
                    ph, wi = call["phase"], call["window"]
                    roff = (region_off + ph * elem
                            + wi * AWIN * (elem * nphase))
                    vw = out[roff:roff + (nrows - 1) * elem * nphase + elem]
                    ov = bass.AP(
                        tensor=vw.tensor, offset=vw.offset,
                        ap=[[elem * nphase, nrows], [1, elem]])
                    if not no_scatter:
                        CH = 896
                        for c0 in range(0, ntot, CH):
                            n_sub = min(CH, ntot - c0)
                            if n_sub not in regcache:
                                regcache[n_sub] = nc.gpsimd.to_reg(n_sub)
                            nc.gpsimd.dma_scatter_add(
                                ov,
                                sums[:, c0 // 128:(c0 + n_sub) // 128, :],
                                it[:, c0 // 16:(c0 + n_sub) // 16],
                                num_idxs=n_sub, num_idxs_reg=regcache[n_sub],
                                elem_size=elem, elem_step=elem * nphase,
                                queue_num=self_state["q"] % 4)
                            self_state["q"] += 1

            if not (no_extras or no_scatter):
                nc.gpsimd.add_instruction(
                    bass_isa.InstPseudoReloadLibraryIndex(
                        name=f"I-{nc.next_id()}", ins=[], outs=[],
                        lib_index=library_config.mlp.index))
            emit_scatter(layout["calls_r"], d_ext, idx_r, 1, NRBF,
                         RPHASES, 0, _emit_radial_terms, "xx")
            emit_scatter(layout["calls_a"], g_ext, idx_a, 8, SUB,
                         APHASES, AOFF, _emit_angular_terms, "xx")

    lower_extended_insts(nc)
    _split_excess_waits(nc, 1)
    return nc


def _split_excess_waits(nc, max_waits=1):
    """This neuronxcc build rejects >1 sem-wait per instruction at codegen;
    hoist extras onto preceding event-semaphore carriers."""
    for f in nc.m.functions:
        for b in f.blocks:
            idx = 0
            while idx < len(b.instructions):
                inst = b.instructions[idx]
                si = inst.sync_info
                if si is not None and len(si.on_wait) > max_waits:
                    waits = list(si.on_wait)
                    keep = waits[-max_waits:]
                    head = waits[:-max_waits]
                    at = idx
                    for i0 in range(0, len(head), max_waits):
                        chunk = head[i0:i0 + max_waits]
                        ev = mybir.InstEventSemaphore(
                            name=nc.get_next_instruction_name(), ins=[],
                            outs=[])
                        ev.engine = inst.engine
                        ev.sync_info = mybir.SyncInfo(on_wait=chunk,
                                                      on_update=[])
                        nc.register_instruction(ev)
                        b.instructions.insert(at, ev)
                        at += 1
                        idx += 1
                    si.on_wait = keep
                    inst.sync_info = si
                idx += 1


# --------------------------------------------------------------------------
# Entry point
# --------------------------------------------------------------------------

_CACHE = {}
LAST_RESULT = {}


def kernel(**inputs):
    import os
    in_maps, layout = _prepare(inputs)
    nc = build_nc(layout)
    trace = os.environ.get("ANI_TRACE") == "1"
    res = run_bass_kernel_spmd(nc, in_maps, core_ids=list(range(NCORE)),
                               trace=trace)
    LAST_RESULT["exec_time_ns"] = getattr(res, "exec_time_ns", None)
    LAST_RESULT["res"] = res
    parts = []
    for c in range(NCORE):
        flat = np.asarray(res.results[c]["out"])
        rad = flat[:AOFF].reshape(NB, S * NRBF)
        ang = flat[AOFF:].reshape(NB, NPAIRS * SUB)
        parts.append(np.concatenate([rad, ang], axis=1))
    return np.concatenate(parts, axis=0).astype(np.float32)
